# revision 7
# baseline (speedup 1.0000x reference)
"""ArcFace loss on 8 TRN2 NeuronCores (Bass/Tile).

Strategy (model-parallel classification head):
  - Classes sharded across 8 cores (12500/core, padded to 12544).
  - Each core:  cosine slice = e_hat @ w_hat_local^T   via TensorEngine,
    with l2-normalization done on-device (norms via squares + ones-matmul
    partition reduce, inv-norm folded into the weight tiles / the exp scale).
  - Row-wise sum of exp(SCALE * cosine) per core (no max-stabilization
    needed: |logits| <= 64 so exp fits comfortably in fp32).
  - AllReduce(add) of the per-row partial sums (4KB) across the 8 cores.
  - The target-class term uses host-gathered weight rows w[labels]
    (replicated, fp32) and the identity cos(acos(x)+m) = x*cos(m) -
    sin(m)*sqrt(1-x^2); every core redundantly computes the final scalar.

kernel(**inputs) takes the FULL inputs and returns the full (scalar) output.
"""

import math

import numpy as np
import ml_dtypes

import concourse.bass as bass
import concourse.mybir as mybir
import concourse.tile as tile
from concourse import bacc

AF = mybir.ActivationFunctionType
ALU = mybir.AluOpType
AX = mybir.AxisListType
F32 = mybir.dt.float32
BF16 = mybir.dt.bfloat16

MARGIN = 0.5
SCALE = 64.0
EPS = 1e-7


def make_cfg(
    n_cores=8,
    b=1024,
    d=512,
    c_total=100000,
    mm_dtype="bf16",
    retain_w=True,
):
    c_local = c_total // n_cores
    c_pad = ((c_local + 127) // 128) * 128
    n_tiles = []
    rem = c_pad
    while rem > 0:
        t = min(512, rem)
        n_tiles.append(t)
        rem -= t
    if mm_dtype == "bf16":
        dt_mm, np_mm = BF16, ml_dtypes.bfloat16
        s_w, s_e, g = 1.0, 1.0, 1.0
        fp8_pairs = False
    elif mm_dtype == "fp8":
        dt_mm, np_mm = mybir.dt.float8e4, ml_dtypes.float8_e4m3
        s_w, s_e, g = 128.0, 1.0, 16.0
        fp8_pairs = True
    elif mm_dtype == "f32":
        dt_mm, np_mm = F32, np.float32
        s_w, s_e, g = 1.0, 1.0, 1.0
        fp8_pairs = False
    else:
        raise ValueError(mm_dtype)
    return dict(
        n_cores=n_cores,
        b=b,
        d=d,
        c_total=c_total,
        c_local=c_local,
        c_pad=c_pad,
        n_tiles=n_tiles,
        dt_mm=dt_mm,
        np_mm=np_mm,
        s_w=s_w,
        s_e=s_e,
        g=g,
        fp8_pairs=fp8_pairs,
        retain_w=retain_w,
        dummy_mag=100.0,
    )


def build_nc(cfg):
    n_cores = cfg["n_cores"]
    b, d = cfg["b"], cfg["d"]
    c_pad = cfg["c_pad"]
    n_tiles = cfg["n_tiles"]
    dt_mm = cfg["dt_mm"]
    g = cfg["g"]
    NT = len(n_tiles)
    KO = d // 128
    BO = b // 128
    P = 128

    nc = bacc.Bacc(
        "TRN2",
        target_bir_lowering=False,
        debug=False,
        enable_asserts=True,
        num_devices=n_cores,
    )

    wt_d = nc.dram_tensor("wt", [P, KO * c_pad], dt_mm, kind="ExternalInput")
    et_d = nc.dram_tensor("et", [P, KO * b], dt_mm, kind="ExternalInput")
    e32_d = nc.dram_tensor("e32", [P, BO * d], F32, kind="ExternalInput")
    wl32_d = nc.dram_tensor("wl32", [P, BO * d], F32, kind="ExternalInput")
    out_d = nc.dram_tensor("out", [1, 1], F32, kind="ExternalOutput")

    cos_m = math.cos(MARGIN)
    sin_m = math.sin(MARGIN)

    with tile.TileContext(nc) as tc:
        with (
            tc.tile_pool(name="const", bufs=1) as pc,
            tc.tile_pool(name="big", bufs=1) as pb,
            tc.tile_pool(name="wpool", bufs=(NT if cfg["retain_w"] else 3)) as pw,
            tc.tile_pool(name="sqpool", bufs=2) as psq,
            tc.tile_pool(name="scr", bufs=4) as pscr,
            tc.tile_pool(name="small", bufs=1) as ps,
            tc.tile_pool(name="ttrs", bufs=2) as pttr,
            tc.tile_pool(name="ps_main", bufs=3, space="PSUM") as ppm,
            tc.tile_pool(name="ps_norm", bufs=2, space="PSUM") as ppn,
            tc.tile_pool(name="ps_e", bufs=1, space="PSUM") as ppe,
            tc.tile_pool(name="ps_fin", bufs=1, space="PSUM") as ppf,
            tc.tile_pool(name="dram", bufs=1, space="DRAM") as pd,
        ):
            # ---- constants ----
            ones_mm = pc.tile([P, P], BF16, tag="ones_mm")
            nc.vector.memset(ones_mm[:], 1.0)
            ones_f = pc.tile([P, 1], F32, tag="ones_f")
            nc.vector.memset(ones_f[:], 1.0)

            # ---- load replicated inputs ----
            et_sb = pb.tile([P, KO, b], dt_mm, tag="et")
            nc.sync.dma_start(
                et_sb[:], et_d.ap().rearrange("p (k b) -> p k b", k=KO)
            )
            e32_sb = pb.tile([P, BO, d], F32, tag="e32")
            nc.sync.dma_start(
                e32_sb[:], e32_d.ap().rearrange("p (o d) -> p o d", o=BO)
            )
            wl32_sb = pb.tile([P, BO, d], F32, tag="wl32")
            nc.sync.dma_start(
                wl32_sb[:], wl32_d.ap().rearrange("p (o d) -> p o d", o=BO)
            )

            # ---- norms of quantized embeddings (for the matmul path) ----
            sq_e = pb.tile([P, KO, b], BF16, tag="sq_e")
            nc.vector.tensor_tensor(sq_e[:], et_sb[:], et_sb[:], ALU.mult)
            ps_e = ppe.tile([P, b], F32, tag="ps_e")
            for h0 in range(0, b, 512):
                hs = slice(h0, min(h0 + 512, b))
                for ko in range(KO):
                    nc.tensor.matmul(
                        ps_e[:, hs],
                        ones_mm[:],
                        sq_e[:, ko, hs],
                        start=(ko == 0),
                        stop=(ko == KO - 1),
                    )
            rne_b = ps.tile([P, b], F32, tag="rne_b")
            nc.vector.reciprocal(rne_b[:], ps_e[:])
            # reshape row 0 (replicated) [1, b] -> [P, BO] with b = bo*128 + p
            # via a DRAM bounce (SBUF partition dim can't be synthesized)
            scale_pre = ps.tile([P, BO], F32, tag="scale_pre")
            nrow = pd.tile([1, b], F32, tag="nrow")
            nc.sync.dma_start(nrow[:], rne_b[0:1, :])
            nc.sync.dma_start(
                scale_pre[:],
                nrow[0:1, :].rearrange("x (o p) -> p (x o)", p=P),
            )
            scale_act = ps.tile([P, BO], F32, tag="scale_act")
            # scale_act = (SCALE/g) / ||e_hat||  = sqrt((SCALE/g)^2 * (1/n2))
            nc.scalar.activation(
                scale_act[:], scale_pre[:], AF.Sqrt, scale=(SCALE / g) ** 2
            )

            # ---- target path (fp32, reference-accurate) ----
            dot = ps.tile([P, BO], F32, tag="dot")
            ne2 = ps.tile([P, BO], F32, tag="ne2")
            nw2 = ps.tile([P, BO], F32, tag="nw2")
            for bo in range(BO):
                for dst, a, bb in (
                    (dot, e32_sb, wl32_sb),
                    (ne2, e32_sb, e32_sb),
                    (nw2, wl32_sb, wl32_sb),
                ):
                    scr = pttr.tile([P, d], F32, tag="ttr")
                    nc.vector.tensor_tensor(
                        scr[:], a[:, bo, :], bb[:, bo, :], ALU.mult
                    )
                    nc.vector.reduce_sum(dst[:, bo : bo + 1], scr[:], axis=AX.X)
            ne2r = ps.tile([P, BO], F32, tag="ne2r")
            nw2r = ps.tile([P, BO], F32, tag="nw2r")
            nc.vector.reciprocal(ne2r[:], ne2[:])
            nc.vector.reciprocal(nw2r[:], nw2[:])
            rne = ps.tile([P, BO], F32, tag="rne")
            rnw = ps.tile([P, BO], F32, tag="rnw")
            nc.scalar.activation(rne[:], ne2r[:], AF.Sqrt)
            nc.scalar.activation(rnw[:], nw2r[:], AF.Sqrt)
            cos_t = ps.tile([P, BO], F32, tag="cos_t")
            nc.vector.tensor_mul(cos_t[:], dot[:], rne[:])
            nc.vector.tensor_mul(cos_t[:], cos_t[:], rnw[:])
            cos_c = ps.tile([P, BO], F32, tag="cos_c")
            nc.vector.tensor_scalar(
                cos_c[:], cos_t[:], 1.0 - EPS, -1.0 + EPS, ALU.min, ALU.max
            )
            cs2 = ps.tile([P, BO], F32, tag="cs2")
            nc.vector.tensor_mul(cs2[:], cos_c[:], cos_c[:])
            sin_t = ps.tile([P, BO], F32, tag="sin_t")
            nc.scalar.activation(sin_t[:], cs2[:], AF.Sqrt, bias=1.0, scale=-1.0)
            tm1 = ps.tile([P, BO], F32, tag="tm1")
            tm2 = ps.tile([P, BO], F32, tag="tm2")
            nc.vector.tensor_scalar_mul(tm1[:], cos_c[:], cos_m)
            nc.vector.tensor_scalar_mul(tm2[:], sin_t[:], sin_m)
            tmod = ps.tile([P, BO], F32, tag="tmod")
            nc.vector.tensor_sub(tmod[:], tm1[:], tm2[:])
            l_m = ps.tile([P, BO], F32, tag="l_m")
            nc.vector.tensor_scalar_mul(l_m[:], tmod[:], SCALE)
            l_t = ps.tile([P, BO], F32, tag="l_t")
            nc.vector.tensor_scalar_mul(l_t[:], cos_t[:], SCALE)

            # ---- pass 1: weight-shard norms ----
            inv_all = pb.tile([P, c_pad], BF16, tag="inv_all")
            w_tiles = []
            c0 = 0
            off = 0
            for ct, nt in enumerate(n_tiles):
                W = pw.tile([P, KO, 512], dt_mm, tag="W")
                nc.sync.dma_start(
                    W[:, :, :nt],
                    wt_d.ap()[:, off : off + KO * nt].rearrange(
                        "p (k n) -> p k n", k=KO
                    ),
                )
                if cfg["retain_w"]:
                    w_tiles.append(W)
                sq = psq.tile([P, KO, 512], BF16, tag="sq")
                nc.vector.tensor_tensor(
                    sq[:, :, :nt], W[:, :, :nt], W[:, :, :nt], ALU.mult
                )
                psn = ppn.tile([P, 512], F32, tag="psn")
                for ko in range(KO):
                    nc.tensor.matmul(
                        psn[:, :nt],
                        ones_mm[:],
                        sq[:, ko, :nt],
                        start=(ko == 0),
                        stop=(ko == KO - 1),
                    )
                with nc.allow_low_precision(reason="1/||w||^2 fits bf16 fine"):
                    nc.vector.reciprocal(inv_all[:, c0 : c0 + nt], psn[:, :nt])
                c0 += nt
                off += KO * nt

            # inv = g / ||W_c|| = sqrt(g^2 * (1/n2)), in place
            nc.scalar.activation(inv_all[:], inv_all[:], AF.Sqrt, scale=g * g)

            # ---- pass 2: scale weights, matmul, exp-accumulate ----
            sums = pb.tile([P, BO, NT], F32, tag="sums")
            c0 = 0
            off = 0
            for ct, nt in enumerate(n_tiles):
                if cfg["retain_w"]:
                    W = w_tiles[ct]
                else:
                    W = pw.tile([P, KO, 512], dt_mm, tag="W")
                    nc.sync.dma_start(
                        W[:, :, :nt],
                        wt_d.ap()[:, off : off + KO * nt].rearrange(
                            "p (k n) -> p k n", k=KO
                        ),
                    )
                inv_b = inv_all[:, None, c0 : c0 + nt].to_broadcast((P, KO, nt))
                nc.vector.tensor_tensor(
                    W[:, :, :nt], W[:, :, :nt], inv_b, ALU.mult
                )
                for bo in range(BO):
                    psm = ppm.tile([P, 512], F32, tag="psm")
                    bs = slice(bo * P, (bo + 1) * P)
                    if cfg["fp8_pairs"]:
                        for kp in range(KO // 2):
                            ks = slice(2 * kp, 2 * kp + 2)
                            nc.tensor.matmul(
                                psm[:, :nt],
                                et_sb[:, ks, bs],
                                W[:, ks, :nt],
                                start=(kp == 0),
                                stop=(kp == KO // 2 - 1),
                                perf_mode=mybir.MatmulPerfMode.DoubleRow,
                            )
                    else:
                        for ko in range(KO):
                            nc.tensor.matmul(
                                psm[:, :nt],
                                et_sb[:, ko, bs],
                                W[:, ko, :nt],
                                start=(ko == 0),
                                stop=(ko == KO - 1),
                            )
                    scr = pscr.tile([P, 512], BF16, tag="escr")
                    nc.scalar.activation(
                        scr[:, :nt],
                        psm[:, :nt],
                        AF.Exp,
                        scale=scale_act[:, bo : bo + 1],
                        accum_out=sums[:, bo, ct : ct + 1],
                    )
                c0 += nt
                off += KO * nt

            # ---- combine: AllReduce of row-wise partial sums ----
            S_loc = ps.tile([P, BO], F32, tag="S_loc")
            nc.vector.reduce_sum(S_loc[:], sums[:], axis=AX.X)
            cc_in = pd.tile([P, BO], F32, tag="cc_in")
            cc_out = pd.tile([P, BO], F32, tag="cc_out")
            nc.gpsimd.dma_start(cc_in[:], S_loc[:])
            nc.gpsimd.collective_compute(
                "AllReduce",
                ALU.add,
                replica_groups=[list(range(n_cores))],
                ins=[cc_in.opt()],
                outs=[cc_out.opt()],
            )
            S_glob = ps.tile([P, BO], F32, tag="S_glob")
            nc.gpsimd.dma_start(S_glob[:], cc_out[:])

            # ---- finalize: S' = S - exp(l_t) + exp(l_m); loss = mean(ln S' - l_m)
            e_lt = ps.tile([P, BO], F32, tag="e_lt")
            e_lm = ps.tile([P, BO], F32, tag="e_lm")
            nc.scalar.activation(e_lt[:], l_t[:], AF.Exp)
            nc.scalar.activation(e_lm[:], l_m[:], AF.Exp)
            S2 = ps.tile([P, BO], F32, tag="S2")
            nc.vector.tensor_sub(S2[:], S_glob[:], e_lt[:])
            nc.vector.tensor_add(S2[:], S2[:], e_lm[:])
            lse = ps.tile([P, BO], F32, tag="lse")
            nc.scalar.activation(lse[:], S2[:], AF.Ln)
            per_b = ps.tile([P, BO], F32, tag="per_b")
            nc.vector.tensor_sub(per_b[:], lse[:], l_m[:])
            row = ps.tile([P, 1], F32, tag="row")
            nc.vector.reduce_sum(row[:], per_b[:], axis=AX.X)
            psf = ppf.tile([1, 1], F32, tag="psf")
            nc.tensor.matmul(psf[:], ones_f[:], row[:], start=True, stop=True)
            loss_sb = ps.tile([1, 1], F32, tag="loss_sb")
            nc.scalar.mul(loss_sb[:], psf[:], 1.0 / b)
            nc.sync.dma_start(out_d.ap()[:], loss_sb[:])

    nc.compile()
    return nc


def prep_inputs(cfg, embeddings, weight, labels):
    """Shard + lay out the full inputs into per-core in_maps."""
    n_cores = cfg["n_cores"]
    b, d = cfg["b"], cfg["d"]
    c_local, c_pad = cfg["c_local"], cfg["c_pad"]
    np_mm = cfg["np_mm"]
    KO = d // 128
    BO = b // 128
    P = 128

    e = np.asarray(embeddings, np.float32)
    w = np.asarray(weight, np.float32)
    lab = np.asarray(labels).astype(np.int64)

    # replicated tensors
    et = (e.T * cfg["s_e"]).astype(np_mm)  # [d, b]
    et_host = np.ascontiguousarray(
        et.reshape(KO, P, b).transpose(1, 0, 2).reshape(P, KO * b)
    )
    e32_host = np.ascontiguousarray(
        e.reshape(BO, P, d).transpose(1, 0, 2).reshape(P, BO * d)
    )
    wl = w[lab]  # [b, d]
    wl32_host = np.ascontiguousarray(
        wl.reshape(BO, P, d).transpose(1, 0, 2).reshape(P, BO * d)
    )

    in_maps = []
    for i in range(n_cores):
        ws = w[i * c_local : (i + 1) * c_local]
        if c_pad > c_local:
            pad = np.zeros((c_pad - c_local, d), np.float32)
            pad[:, 0] = cfg["dummy_mag"] / cfg["s_w"]
            ws = np.concatenate([ws, pad], axis=0)
        wt = (ws.T * cfg["s_w"]).astype(np_mm)  # [d, c_pad]
        wt4 = wt.reshape(KO, P, c_pad)  # [ko, p, c]
        blocks = []
        c0 = 0
        for nt in cfg["n_tiles"]:
            blk = wt4[:, :, c0 : c0 + nt]  # [KO, P, nt]
            blocks.append(blk.transpose(1, 0, 2).reshape(P, KO * nt))
            c0 += nt
        wt_host = np.ascontiguousarray(np.concatenate(blocks, axis=1))
        in_maps.append(
            {
                "wt": wt_host,
                "et": et_host,
                "e32": e32_host,
                "wl32": wl32_host,
            }
        )
    return in_maps


_CACHED = {}


def _get_nc(cfg_key, cfg):
    if cfg_key not in _CACHED:
        _CACHED[cfg_key] = build_nc(cfg)
    return _CACHED[cfg_key]


def run(inputs, mm_dtype="fp8", retain_w=True, trace=False, **kw):
    from concourse.bass_utils import run_bass_kernel_spmd

    cfg = make_cfg(mm_dtype=mm_dtype, retain_w=retain_w)
    nc = _get_nc((mm_dtype, retain_w), cfg)
    in_maps = prep_inputs(
        cfg, inputs["embeddings"], inputs["weight"], inputs["labels"]
    )
    res = run_bass_kernel_spmd(
        nc, in_maps, core_ids=list(range(cfg["n_cores"])), trace=trace, **kw
    )
    loss = np.float32(res.results[0]["out"].reshape(-1)[0])
    return loss, res


def kernel(**inputs):
    loss, _ = run(inputs, trace=False)
    return np.asarray(loss, dtype=np.float32).reshape(())


# revision 8
# speedup vs baseline: 1.0821x; 1.0821x over previous
"""ArcFace loss on 8 TRN2 NeuronCores (Bass/Tile).

Strategy (model-parallel classification head):
  - Classes sharded across 8 cores (12500/core, padded to 12544).
  - Each core: cosine slice = e_hat @ w_hat_local^T on the TensorEngine
    (fp8 DoubleRow by default), with l2-normalization done on-device:
    per-class norms come from a one-pass DVE/ACT square-accumulate over a
    row-major copy of the shard, the inverse norms are folded into the
    weight tiles, and 1/||e|| is folded into the exp scale.
  - Row-wise sum of exp(SCALE * cosine): ACT exp with accum_out over
    2048-column PSUM groups (no max-stabilization needed: |logits| <= 64
    so exp fits comfortably in fp32).
  - AllReduce(add) of the per-row partial sums (4KB) across the 8 cores.
  - Target-class terms use host-gathered rows w[labels] (replicated,
    fp32) and cos(acos(x)+m) = x*cos(m) - sin(m)*sqrt(1-x^2); every core
    redundantly computes the final scalar.

kernel(**inputs) takes the FULL inputs and returns the full (scalar) output.
"""

import math

import numpy as np
import ml_dtypes

import concourse.bass as bass
import concourse.mybir as mybir
import concourse.tile as tile
from concourse import bacc

AF = mybir.ActivationFunctionType
ALU = mybir.AluOpType
AX = mybir.AxisListType
F32 = mybir.dt.float32
BF16 = mybir.dt.bfloat16

MARGIN = 0.5
SCALE = 64.0
EPS = 1e-7


def make_cfg(
    n_cores=8,
    b=1024,
    d=512,
    c_total=100000,
    mm_dtype="fp8",
):
    c_local = c_total // n_cores
    c_pad = ((c_local + 127) // 128) * 128
    n_tiles = []
    rem = c_pad
    while rem > 0:
        t = min(512, rem)
        n_tiles.append(t)
        rem -= t
    # groups of up to 4 column tiles -> one 2048-wide exp per group
    groups = []
    i = 0
    while i < len(n_tiles):
        g = list(range(i, min(i + 4, len(n_tiles))))
        if sum(n_tiles[j] for j in g) > 2048:
            g = g[:-1]
        groups.append(g)
        i = g[-1] + 1
    if mm_dtype == "bf16":
        dt_mm, np_mm = BF16, ml_dtypes.bfloat16
        s_w, s_e, g = 1.0, 1.0, 1.0
        fp8_pairs = False
    elif mm_dtype == "fp8":
        dt_mm, np_mm = mybir.dt.float8e4, ml_dtypes.float8_e4m3
        s_w, s_e, g = 128.0, 1.0, 16.0
        fp8_pairs = True
    else:
        raise ValueError(mm_dtype)
    return dict(
        n_cores=n_cores,
        b=b,
        d=d,
        c_total=c_total,
        c_local=c_local,
        c_pad=c_pad,
        n_tiles=n_tiles,
        groups=groups,
        dt_mm=dt_mm,
        np_mm=np_mm,
        s_w=s_w,
        s_e=s_e,
        g=g,
        fp8_pairs=fp8_pairs,
        dummy_mag=100.0,
    )


def build_nc(cfg):
    n_cores = cfg["n_cores"]
    b, d = cfg["b"], cfg["d"]
    c_pad = cfg["c_pad"]
    n_tiles = cfg["n_tiles"]
    groups = cfg["groups"]
    dt_mm = cfg["dt_mm"]
    g = cfg["g"]
    NT = len(n_tiles)
    NG = len(groups)
    KO = d // 128
    BO = b // 128
    JP = c_pad // 128
    P = 128

    nc = bacc.Bacc(
        "TRN2",
        target_bir_lowering=False,
        debug=False,
        enable_asserts=True,
        num_devices=n_cores,
    )

    wt_d = nc.dram_tensor("wt", [P, KO * c_pad], dt_mm, kind="ExternalInput")
    wr_d = nc.dram_tensor("wr", [P, JP * d], dt_mm, kind="ExternalInput")
    et_d = nc.dram_tensor("et", [P, KO * b], dt_mm, kind="ExternalInput")
    e32_d = nc.dram_tensor("e32", [P, BO * d], F32, kind="ExternalInput")
    wl32_d = nc.dram_tensor("wl32", [P, BO * d], F32, kind="ExternalInput")
    out_d = nc.dram_tensor("out", [1, 1], F32, kind="ExternalOutput")

    cos_m = math.cos(MARGIN)
    sin_m = math.sin(MARGIN)

    with tile.TileContext(nc) as tc:
        with (
            tc.tile_pool(name="const", bufs=1) as pc,
            tc.tile_pool(name="big", bufs=1) as pb,
            tc.tile_pool(name="wpool", bufs=NT) as pw,
            tc.tile_pool(name="wrpool", bufs=6) as pwr,
            tc.tile_pool(name="scr", bufs=3) as pscr,
            tc.tile_pool(name="nscr", bufs=4) as pnscr,
            tc.tile_pool(name="small", bufs=1) as ps,
            tc.tile_pool(name="ttrs", bufs=2) as pttr,
            tc.tile_pool(name="ps_all", bufs=2, space="PSUM") as pps,
            tc.tile_pool(name="dram", bufs=1, space="DRAM") as pd,
        ):
            # ---- constants ----
            ones_mm = pc.tile([P, P], BF16, tag="ones_mm")
            nc.vector.memset(ones_mm[:], 1.0)
            ones_f = pc.tile([P, 1], F32, tag="ones_f")
            nc.vector.memset(ones_f[:], 1.0)

            # ---- load replicated inputs ----
            et_sb = pb.tile([P, KO, b], dt_mm, tag="et")
            nc.sync.dma_start(
                et_sb[:], et_d.ap().rearrange("p (k b) -> p k b", k=KO)
            )
            e32_sb = pb.tile([P, BO, d], F32, tag="e32")
            nc.sync.dma_start(
                e32_sb[:], e32_d.ap().rearrange("p (o d) -> p o d", o=BO)
            )
            wl32_sb = pb.tile([P, BO, d], F32, tag="wl32")
            nc.sync.dma_start(
                wl32_sb[:], wl32_d.ap().rearrange("p (o d) -> p o d", o=BO)
            )

            # ---- norms of quantized embeddings (for the matmul path) ----
            sq_e = pb.tile([P, KO, b], BF16, tag="sq_e")
            nc.vector.tensor_tensor(sq_e[:], et_sb[:], et_sb[:], ALU.mult)
            ps_e = pps.tile([P, b], F32, tag="ps")
            for h0 in range(0, b, 512):
                hs = slice(h0, min(h0 + 512, b))
                for ko in range(KO):
                    nc.tensor.matmul(
                        ps_e[:, hs],
                        ones_mm[:],
                        sq_e[:, ko, hs],
                        start=(ko == 0),
                        stop=(ko == KO - 1),
                    )
            rne_b = ps.tile([P, b], F32, tag="rne_b")
            nc.vector.reciprocal(rne_b[:], ps_e[:])
            # reshape row 0 (replicated) [1, b] -> [P, BO] with b = bo*128 + p
            # via a DRAM bounce (SBUF partition dim can't be synthesized)
            scale_pre = ps.tile([P, BO], F32, tag="scale_pre")
            nrow = pd.tile([1, b], F32, tag="nrow")
            nc.sync.dma_start(nrow[:], rne_b[0:1, :])
            nc.sync.dma_start(
                scale_pre[:],
                nrow[0:1, :].rearrange("x (o p) -> p (x o)", p=P),
            )
            scale_act = ps.tile([P, BO], F32, tag="scale_act")
            # scale_act = (SCALE/g) / ||e_hat||  = sqrt((SCALE/g)^2 * (1/n2))
            nc.scalar.activation(
                scale_act[:], scale_pre[:], AF.Sqrt, scale=(SCALE / g) ** 2
            )

            # ---- target path (fp32, reference-accurate) ----
            dot = ps.tile([P, BO], F32, tag="dot")
            ne2 = ps.tile([P, BO], F32, tag="ne2")
            nw2 = ps.tile([P, BO], F32, tag="nw2")
            for bo in range(BO):
                for dst, a, bb in (
                    (dot, e32_sb, wl32_sb),
                    (ne2, e32_sb, e32_sb),
                    (nw2, wl32_sb, wl32_sb),
                ):
                    scr = pttr.tile([P, d], F32, tag="ttr")
                    nc.vector.scalar_tensor_tensor(
                        out=scr[:],
                        in0=a[:, bo, :],
                        scalar=1.0,
                        in1=bb[:, bo, :],
                        op0=ALU.mult,
                        op1=ALU.mult,
                        accum_out=dst[:, bo : bo + 1],
                    )
            ne2r = ps.tile([P, BO], F32, tag="ne2r")
            nw2r = ps.tile([P, BO], F32, tag="nw2r")
            nc.vector.reciprocal(ne2r[:], ne2[:])
            nc.vector.reciprocal(nw2r[:], nw2[:])
            rne = ps.tile([P, BO], F32, tag="rne")
            rnw = ps.tile([P, BO], F32, tag="rnw")
            nc.scalar.activation(rne[:], ne2r[:], AF.Sqrt)
            nc.scalar.activation(rnw[:], nw2r[:], AF.Sqrt)
            cos_t = ps.tile([P, BO], F32, tag="cos_t")
            nc.vector.tensor_mul(cos_t[:], dot[:], rne[:])
            nc.vector.tensor_mul(cos_t[:], cos_t[:], rnw[:])
            cos_c = ps.tile([P, BO], F32, tag="cos_c")
            nc.vector.tensor_scalar(
                cos_c[:], cos_t[:], 1.0 - EPS, -1.0 + EPS, ALU.min, ALU.max
            )
            cs2 = ps.tile([P, BO], F32, tag="cs2")
            nc.vector.tensor_mul(cs2[:], cos_c[:], cos_c[:])
            sin_t = ps.tile([P, BO], F32, tag="sin_t")
            nc.scalar.activation(sin_t[:], cs2[:], AF.Sqrt, bias=1.0, scale=-1.0)
            tm1 = ps.tile([P, BO], F32, tag="tm1")
            tm2 = ps.tile([P, BO], F32, tag="tm2")
            nc.vector.tensor_scalar_mul(tm1[:], cos_c[:], cos_m)
            nc.vector.tensor_scalar_mul(tm2[:], sin_t[:], sin_m)
            tmod = ps.tile([P, BO], F32, tag="tmod")
            nc.vector.tensor_sub(tmod[:], tm1[:], tm2[:])
            l_m = ps.tile([P, BO], F32, tag="l_m")
            nc.vector.tensor_scalar_mul(l_m[:], tmod[:], SCALE)
            l_t = ps.tile([P, BO], F32, tag="l_t")
            nc.vector.tensor_scalar_mul(l_t[:], cos_t[:], SCALE)

            # ---- pass 1a: DMA the w^T tiles used by the matmuls ----
            w_tiles = []
            off = 0
            for ct, nt in enumerate(n_tiles):
                W = pw.tile([P, KO, 512], dt_mm, tag="W")
                nc.sync.dma_start(
                    W[:, :, :nt],
                    wt_d.ap()[:, off : off + KO * nt].rearrange(
                        "p (k n) -> p k n", k=KO
                    ),
                )
                w_tiles.append(W)
                off += KO * nt

            # ---- pass 1b: per-class norms via square-accumulate over the
            # row-major shard copy (compact [P, JP] layout, c = p*JP + j) ----
            nsq = ps.tile([P, JP], F32, tag="nsq")
            j = 0
            wg = 0
            while j < JP:
                gn = min(4, JP - j)
                wr_t = pwr.tile([P, 4, d], dt_mm, tag="wr")
                nc.sync.dma_start(
                    wr_t[:, :gn, :],
                    wr_d.ap()[:, j * d : (j + gn) * d].rearrange(
                        "p (j dd) -> p j dd", j=gn
                    ),
                )
                for jj in range(gn):
                    if (j + jj) % 10 < 3:
                        nscr = pnscr.tile([P, d], BF16, tag="nscr")
                        nc.scalar.activation(
                            nscr[:],
                            wr_t[:, jj, :],
                            AF.Square,
                            accum_out=nsq[:, j + jj : j + jj + 1],
                        )
                    else:
                        nscr = pnscr.tile([P, d], BF16, tag="nscr")
                        nc.vector.scalar_tensor_tensor(
                            out=nscr[:],
                            in0=wr_t[:, jj, :],
                            scalar=1.0,
                            in1=wr_t[:, jj, :],
                            op0=ALU.mult,
                            op1=ALU.mult,
                            accum_out=nsq[:, j + jj : j + jj + 1],
                        )
                j += gn
                wg += 1

            # inv_cc = g / ||W_c|| = sqrt(g^2 / n2), compact bf16
            nsqr = ps.tile([P, JP], F32, tag="nsqr")
            nc.vector.reciprocal(nsqr[:], nsq[:])
            inv_cc = ps.tile([P, JP], BF16, tag="inv_cc")
            nc.scalar.activation(inv_cc[:], nsqr[:], AF.Sqrt, scale=g * g)

            # compact [P, JP] -> DRAM row in class order (contiguous per
            # partition since c = p*JP + j) -> partition-broadcast back
            invrow = pd.tile([1, c_pad], BF16, tag="invrow")
            nc.sync.dma_start(
                invrow[0:1, :].rearrange("x (p j) -> p (x j)", p=P),
                inv_cc[:],
            )
            inv_bc = pb.tile([P, c_pad], BF16, tag="inv_bc")
            bc_ap = bass.AP(
                tensor=invrow.tensor,
                offset=invrow.offset,
                ap=[[0, P], [1, c_pad]],
            )
            nc.gpsimd.dma_start(inv_bc[:], bc_ap)

            # ---- scale the w^T tiles in place by inv-norm ----
            c0 = 0
            for ct, nt in enumerate(n_tiles):
                W = w_tiles[ct]
                inv_b = inv_bc[:, None, c0 : c0 + nt].to_broadcast((P, KO, nt))
                nc.vector.tensor_tensor(
                    W[:, :, :nt], W[:, :, :nt], inv_b, ALU.mult
                )
                c0 += nt

            # ---- pass 2: matmul + exp-accumulate over 2048-col groups ----
            sums = pb.tile([P, BO, NG], F32, tag="sums")
            tile_off = [0]
            for nt in n_tiles:
                tile_off.append(tile_off[-1] + nt)
            for bo in range(BO):
                bs = slice(bo * P, (bo + 1) * P)
                for gi, grp in enumerate(groups):
                    gw = sum(n_tiles[ct] for ct in grp)
                    psm = pps.tile([P, 2048], F32, tag="ps")
                    if cfg["fp8_pairs"]:
                        for kp in range(KO // 2):
                            ks = slice(2 * kp, 2 * kp + 2)
                            o = 0
                            for ct in grp:
                                nt = n_tiles[ct]
                                nc.tensor.matmul(
                                    psm[:, o : o + nt],
                                    et_sb[:, ks, bs],
                                    w_tiles[ct][:, ks, :nt],
                                    start=(kp == 0),
                                    stop=(kp == KO // 2 - 1),
                                    perf_mode=mybir.MatmulPerfMode.DoubleRow,
                                )
                                o += nt
                    else:
                        for ko in range(KO):
                            o = 0
                            for ct in grp:
                                nt = n_tiles[ct]
                                nc.tensor.matmul(
                                    psm[:, o : o + nt],
                                    et_sb[:, ko, bs],
                                    w_tiles[ct][:, ko, :nt],
                                    start=(ko == 0),
                                    stop=(ko == KO - 1),
                                )
                                o += nt
                    scr = pscr.tile([P, 2048], BF16, tag="escr")
                    nc.scalar.activation(
                        scr[:, :gw],
                        psm[:, :gw],
                        AF.Exp,
                        scale=scale_act[:, bo : bo + 1],
                        accum_out=sums[:, bo, gi : gi + 1],
                    )

            # ---- combine: AllReduce of row-wise partial sums ----
            S_loc = ps.tile([P, BO], F32, tag="S_loc")
            nc.vector.reduce_sum(S_loc[:], sums[:], axis=AX.X)
            cc_in = pd.tile([P, BO], F32, tag="cc_in")
            cc_out = pd.tile([P, BO], F32, tag="cc_out")
            nc.gpsimd.dma_start(cc_in[:], S_loc[:])
            nc.gpsimd.collective_compute(
                "AllReduce",
                ALU.add,
                replica_groups=[list(range(n_cores))],
                ins=[cc_in.opt()],
                outs=[cc_out.opt()],
            )
            S_glob = ps.tile([P, BO], F32, tag="S_glob")
            nc.gpsimd.dma_start(S_glob[:], cc_out[:])

            # ---- finalize: S' = S - exp(l_t) + exp(l_m); loss = mean(ln S' - l_m)
            e_lt = ps.tile([P, BO], F32, tag="e_lt")
            e_lm = ps.tile([P, BO], F32, tag="e_lm")
            nc.scalar.activation(e_lt[:], l_t[:], AF.Exp)
            nc.scalar.activation(e_lm[:], l_m[:], AF.Exp)
            S2 = ps.tile([P, BO], F32, tag="S2")
            nc.vector.tensor_sub(S2[:], S_glob[:], e_lt[:])
            nc.vector.tensor_add(S2[:], S2[:], e_lm[:])
            lse = ps.tile([P, BO], F32, tag="lse")
            nc.scalar.activation(lse[:], S2[:], AF.Ln)
            per_b = ps.tile([P, BO], F32, tag="per_b")
            nc.vector.tensor_sub(per_b[:], lse[:], l_m[:])
            row = ps.tile([P, 1], F32, tag="row")
            nc.vector.reduce_sum(row[:], per_b[:], axis=AX.X)
            psf = pps.tile([1, 1], F32, tag="ps")
            nc.tensor.matmul(psf[:], ones_f[:], row[:], start=True, stop=True)
            loss_sb = ps.tile([1, 1], F32, tag="loss_sb")
            nc.scalar.mul(loss_sb[:], psf[:], 1.0 / b)
            nc.sync.dma_start(out_d.ap()[:], loss_sb[:])

    nc.compile()
    return nc


def prep_inputs(cfg, embeddings, weight, labels):
    """Shard + lay out the full inputs into per-core in_maps."""
    n_cores = cfg["n_cores"]
    b, d = cfg["b"], cfg["d"]
    c_local, c_pad = cfg["c_local"], cfg["c_pad"]
    np_mm = cfg["np_mm"]
    KO = d // 128
    BO = b // 128
    JP = c_pad // 128
    P = 128

    e = np.asarray(embeddings, np.float32)
    w = np.asarray(weight, np.float32)
    lab = np.asarray(labels).astype(np.int64)

    # replicated tensors
    et = (e.T * cfg["s_e"]).astype(np_mm)  # [d, b]
    et_host = np.ascontiguousarray(
        et.reshape(KO, P, b).transpose(1, 0, 2).reshape(P, KO * b)
    )
    e32_host = np.ascontiguousarray(
        e.reshape(BO, P, d).transpose(1, 0, 2).reshape(P, BO * d)
    )
    wl = w[lab]  # [b, d]
    wl32_host = np.ascontiguousarray(
        wl.reshape(BO, P, d).transpose(1, 0, 2).reshape(P, BO * d)
    )

    in_maps = []
    for i in range(n_cores):
        ws = w[i * c_local : (i + 1) * c_local]
        if c_pad > c_local:
            pad = np.zeros((c_pad - c_local, d), np.float32)
            pad[:, 0] = cfg["dummy_mag"] / cfg["s_w"]
            ws = np.concatenate([ws, pad], axis=0)
        ws_scaled = (ws * cfg["s_w"]).astype(np_mm)  # [c_pad, d]
        wt = ws_scaled.T  # [d, c_pad]
        wt4 = np.ascontiguousarray(wt).reshape(KO, P, c_pad)  # [ko, p, c]
        blocks = []
        c0 = 0
        for nt in cfg["n_tiles"]:
            blk = wt4[:, :, c0 : c0 + nt]  # [KO, P, nt]
            blocks.append(blk.transpose(1, 0, 2).reshape(P, KO * nt))
            c0 += nt
        wt_host = np.ascontiguousarray(np.concatenate(blocks, axis=1))
        # row-major copy for norms: partition p holds classes [p*JP,(p+1)*JP)
        wr_host = np.ascontiguousarray(ws_scaled.reshape(P, JP * d))
        in_maps.append(
            {
                "wt": wt_host,
                "wr": wr_host,
                "et": et_host,
                "e32": e32_host,
                "wl32": wl32_host,
            }
        )
    return in_maps


_CACHED = {}


def _get_nc(cfg_key, cfg):
    if cfg_key not in _CACHED:
        _CACHED[cfg_key] = build_nc(cfg)
    return _CACHED[cfg_key]


def run(inputs, mm_dtype="fp8", trace=False, **kw):
    from concourse.bass_utils import run_bass_kernel_spmd

    cfg = make_cfg(mm_dtype=mm_dtype)
    nc = _get_nc((mm_dtype,), cfg)
    in_maps = prep_inputs(
        cfg, inputs["embeddings"], inputs["weight"], inputs["labels"]
    )
    res = run_bass_kernel_spmd(
        nc, in_maps, core_ids=list(range(cfg["n_cores"])), trace=trace, **kw
    )
    loss = np.float32(res.results[0]["out"].reshape(-1)[0])
    return loss, res


def kernel(**inputs):
    loss, _ = run(inputs, trace=False)
    return np.asarray(loss, dtype=np.float32).reshape(())


# revision 11
# speedup vs baseline: 1.3037x; 1.2048x over previous
"""ArcFace loss on 8 TRN2 NeuronCores (Bass/Tile).

Strategy (model-parallel classification head):
  - Classes sharded across 8 cores (12500/core, padded to 12544).
  - Each core: cosine slice = e_hat @ w_hat_local^T on the TensorEngine
    (fp8 DoubleRow by default), with l2-normalization done on-device:
    per-class norms come from a one-pass DVE/ACT square-accumulate over a
    row-major copy of the shard, the inverse norms are folded into the
    weight tiles, and 1/||e|| is folded into the exp scale.
  - Row-wise sum of exp(SCALE * cosine): ACT exp with accum_out over
    2048-column PSUM groups (no max-stabilization needed: |logits| <= 64
    so exp fits comfortably in fp32).
  - AllReduce(add) of the per-row partial sums (4KB) across the 8 cores.
  - Target-class terms use host-gathered rows w[labels] (replicated,
    fp32) and cos(acos(x)+m) = x*cos(m) - sin(m)*sqrt(1-x^2); every core
    redundantly computes the final scalar.

kernel(**inputs) takes the FULL inputs and returns the full (scalar) output.
"""

import math

import numpy as np
import ml_dtypes

import concourse.bass as bass
import concourse.mybir as mybir
import concourse.tile as tile
from concourse import bacc

AF = mybir.ActivationFunctionType
ALU = mybir.AluOpType
AX = mybir.AxisListType
F32 = mybir.dt.float32
BF16 = mybir.dt.bfloat16

MARGIN = 0.5
SCALE = 64.0
EPS = 1e-7


def make_cfg(
    n_cores=8,
    b=1024,
    d=512,
    c_total=100000,
    mm_dtype="fp8",
):
    c_local = c_total // n_cores
    c_pad = ((c_local + 127) // 128) * 128
    n_tiles = []
    rem = c_pad
    while rem > 0:
        t = min(512, rem)
        n_tiles.append(t)
        rem -= t
    # groups of up to 4 column tiles -> one 2048-wide exp per group
    groups = []
    i = 0
    while i < len(n_tiles):
        g = list(range(i, min(i + 4, len(n_tiles))))
        if sum(n_tiles[j] for j in g) > 2048:
            g = g[:-1]
        groups.append(g)
        i = g[-1] + 1
    if mm_dtype == "bf16":
        dt_mm, np_mm = BF16, ml_dtypes.bfloat16
        s_w, s_e, g = 1.0, 1.0, 1.0
        fp8_pairs = False
    elif mm_dtype == "fp8":
        dt_mm, np_mm = mybir.dt.float8e4, ml_dtypes.float8_e4m3
        s_w, s_e, g = 128.0, 1.0, 16.0
        fp8_pairs = True
    else:
        raise ValueError(mm_dtype)
    return dict(
        n_cores=n_cores,
        b=b,
        d=d,
        c_total=c_total,
        c_local=c_local,
        c_pad=c_pad,
        n_tiles=n_tiles,
        groups=groups,
        dt_mm=dt_mm,
        np_mm=np_mm,
        s_w=s_w,
        s_e=s_e,
        g=g,
        fp8_pairs=fp8_pairs,
        dummy_mag=100.0,
    )


def build_nc(cfg):
    n_cores = cfg["n_cores"]
    b, d = cfg["b"], cfg["d"]
    c_pad = cfg["c_pad"]
    n_tiles = cfg["n_tiles"]
    groups = cfg["groups"]
    dt_mm = cfg["dt_mm"]
    g = cfg["g"]
    NT = len(n_tiles)
    NG = len(groups)
    KO = d // 128
    BO = b // 128
    JP = c_pad // 128
    P = 128

    nc = bacc.Bacc(
        "TRN2",
        target_bir_lowering=False,
        debug=False,
        enable_asserts=True,
        num_devices=n_cores,
    )

    wt_d = nc.dram_tensor("wt", [P, KO * c_pad], dt_mm, kind="ExternalInput")
    wr_d = nc.dram_tensor("wr", [P, JP * d], dt_mm, kind="ExternalInput")
    et_d = nc.dram_tensor("et", [P, KO * b], dt_mm, kind="ExternalInput")
    e32_d = nc.dram_tensor("e32", [P, BO * d], F32, kind="ExternalInput")
    wl32_d = nc.dram_tensor("wl32", [P, BO * d], F32, kind="ExternalInput")
    out_d = nc.dram_tensor("out", [1, 1], F32, kind="ExternalOutput")

    cos_m = math.cos(MARGIN)
    sin_m = math.sin(MARGIN)

    with tile.TileContext(nc) as tc:
        with (
            tc.tile_pool(name="const", bufs=1) as pc,
            tc.tile_pool(name="big", bufs=1) as pb,
            tc.tile_pool(name="wpool", bufs=3) as pw,
            tc.tile_pool(name="wrpool", bufs=6) as pwr,
            tc.tile_pool(name="scr", bufs=3) as pscr,
            tc.tile_pool(name="nscr", bufs=4) as pnscr,
            tc.tile_pool(name="small", bufs=1) as ps,
            tc.tile_pool(name="ttrs", bufs=2) as pttr,
            tc.tile_pool(name="ps_all", bufs=2, space="PSUM") as pps,
            tc.tile_pool(name="dram", bufs=1, space="DRAM") as pd,
        ):
            # ---- constants ----
            ones_mm = pc.tile([P, P], BF16, tag="ones_mm")
            nc.vector.memset(ones_mm[:], 1.0)
            ones_f = pc.tile([P, 1], F32, tag="ones_f")
            nc.vector.memset(ones_f[:], 1.0)

            # ---- load replicated inputs ----
            et_sb = pb.tile([P, KO, b], dt_mm, tag="et")
            nc.sync.dma_start(
                et_sb[:], et_d.ap().rearrange("p (k b) -> p k b", k=KO)
            )
            e32_sb = pb.tile([P, BO, d], F32, tag="e32")
            nc.sync.dma_start(
                e32_sb[:], e32_d.ap().rearrange("p (o d) -> p o d", o=BO)
            )
            wl32_sb = pb.tile([P, BO, d], F32, tag="wl32")
            nc.sync.dma_start(
                wl32_sb[:], wl32_d.ap().rearrange("p (o d) -> p o d", o=BO)
            )

            # ---- norms of quantized embeddings (for the matmul path) ----
            sq_e = pb.tile([P, KO, b], BF16, tag="sq_e")
            nc.vector.tensor_tensor(sq_e[:], et_sb[:], et_sb[:], ALU.mult)
            ps_e = pps.tile([P, b], F32, tag="ps")
            for h0 in range(0, b, 512):
                hs = slice(h0, min(h0 + 512, b))
                for ko in range(KO):
                    nc.tensor.matmul(
                        ps_e[:, hs],
                        ones_mm[:],
                        sq_e[:, ko, hs],
                        start=(ko == 0),
                        stop=(ko == KO - 1),
                    )
            rne_b = ps.tile([P, b], F32, tag="rne_b")
            nc.vector.reciprocal(rne_b[:], ps_e[:])
            # reshape row 0 (replicated) [1, b] -> [P, BO] with b = bo*128 + p
            # via a DRAM bounce (SBUF partition dim can't be synthesized)
            scale_pre = ps.tile([P, BO], F32, tag="scale_pre")
            nrow = pd.tile([1, b], F32, tag="nrow")
            nc.sync.dma_start(nrow[:], rne_b[0:1, :])
            nc.sync.dma_start(
                scale_pre[:],
                nrow[0:1, :].rearrange("x (o p) -> p (x o)", p=P),
            )
            scale_act = ps.tile([P, BO], F32, tag="scale_act")
            # scale_act = (SCALE/g) / ||e_hat||  = sqrt((SCALE/g)^2 * (1/n2))
            nc.scalar.activation(
                scale_act[:], scale_pre[:], AF.Sqrt, scale=(SCALE / g) ** 2
            )

            # ---- target path (fp32, reference-accurate) ----
            dot = ps.tile([P, BO], F32, tag="dot")
            ne2 = ps.tile([P, BO], F32, tag="ne2")
            nw2 = ps.tile([P, BO], F32, tag="nw2")
            for bo in range(BO):
                for dst, a, bb in (
                    (dot, e32_sb, wl32_sb),
                    (ne2, e32_sb, e32_sb),
                    (nw2, wl32_sb, wl32_sb),
                ):
                    scr = pttr.tile([P, d], F32, tag="ttr")
                    nc.vector.scalar_tensor_tensor(
                        out=scr[:],
                        in0=a[:, bo, :],
                        scalar=1.0,
                        in1=bb[:, bo, :],
                        op0=ALU.mult,
                        op1=ALU.mult,
                        accum_out=dst[:, bo : bo + 1],
                    )
            ne2r = ps.tile([P, BO], F32, tag="ne2r")
            nw2r = ps.tile([P, BO], F32, tag="nw2r")
            nc.vector.reciprocal(ne2r[:], ne2[:])
            nc.vector.reciprocal(nw2r[:], nw2[:])
            rne = ps.tile([P, BO], F32, tag="rne")
            rnw = ps.tile([P, BO], F32, tag="rnw")
            nc.scalar.activation(rne[:], ne2r[:], AF.Sqrt)
            nc.scalar.activation(rnw[:], nw2r[:], AF.Sqrt)
            cos_t = ps.tile([P, BO], F32, tag="cos_t")
            nc.vector.tensor_mul(cos_t[:], dot[:], rne[:])
            nc.vector.tensor_mul(cos_t[:], cos_t[:], rnw[:])
            cos_c = ps.tile([P, BO], F32, tag="cos_c")
            nc.vector.tensor_scalar(
                cos_c[:], cos_t[:], 1.0 - EPS, -1.0 + EPS, ALU.min, ALU.max
            )
            cs2 = ps.tile([P, BO], F32, tag="cs2")
            nc.vector.tensor_mul(cs2[:], cos_c[:], cos_c[:])
            sin_t = ps.tile([P, BO], F32, tag="sin_t")
            nc.scalar.activation(sin_t[:], cs2[:], AF.Sqrt, bias=1.0, scale=-1.0)
            tm1 = ps.tile([P, BO], F32, tag="tm1")
            tm2 = ps.tile([P, BO], F32, tag="tm2")
            nc.vector.tensor_scalar_mul(tm1[:], cos_c[:], cos_m)
            nc.vector.tensor_scalar_mul(tm2[:], sin_t[:], sin_m)
            tmod = ps.tile([P, BO], F32, tag="tmod")
            nc.vector.tensor_sub(tmod[:], tm1[:], tm2[:])
            l_m = ps.tile([P, BO], F32, tag="l_m")
            nc.vector.tensor_scalar_mul(l_m[:], tmod[:], SCALE)
            l_t = ps.tile([P, BO], F32, tag="l_t")
            nc.vector.tensor_scalar_mul(l_t[:], cos_t[:], SCALE)

            # ---- pass 1: per-class norms via square-accumulate over the
            # row-major shard copy (compact [P, JP] layout, c = p*JP + j) ----
            nsq = ps.tile([P, JP], F32, tag="nsq")
            j = 0
            wg = 0
            while j < JP:
                gn = min(4, JP - j)
                wr_t = pwr.tile([P, 4, d], dt_mm, tag="wr")
                nc.sync.dma_start(
                    wr_t[:, :gn, :],
                    wr_d.ap()[:, j * d : (j + gn) * d].rearrange(
                        "p (j dd) -> p j dd", j=gn
                    ),
                )
                for jj in range(gn):
                    if (j + jj) % 7 < 3:
                        nscr = pnscr.tile([P, d], BF16, tag="nscr")
                        nc.scalar.activation(
                            nscr[:],
                            wr_t[:, jj, :],
                            AF.Square,
                            accum_out=nsq[:, j + jj : j + jj + 1],
                        )
                    else:
                        nscr = pnscr.tile([P, d], BF16, tag="nscr")
                        nc.vector.scalar_tensor_tensor(
                            out=nscr[:],
                            in0=wr_t[:, jj, :],
                            scalar=1.0,
                            in1=wr_t[:, jj, :],
                            op0=ALU.mult,
                            op1=ALU.mult,
                            accum_out=nsq[:, j + jj : j + jj + 1],
                        )
                j += gn
                wg += 1

            # inv_cc = g / ||W_c|| = sqrt(g^2 / n2), compact bf16
            nsqr = ps.tile([P, JP], F32, tag="nsqr")
            nc.vector.reciprocal(nsqr[:], nsq[:])
            inv_cc = ps.tile([P, JP], BF16, tag="inv_cc")
            nc.scalar.activation(inv_cc[:], nsqr[:], AF.Sqrt, scale=g * g)

            # compact [P, JP] -> DRAM row in class order (contiguous per
            # partition since c = p*JP + j) -> partition-broadcast back
            invrow = pd.tile([1, c_pad], BF16, tag="invrow")
            nc.sync.dma_start(
                invrow[0:1, :].rearrange("x (p j) -> p (x j)", p=P),
                inv_cc[:],
            )
            inv_bc = pb.tile([P, c_pad], BF16, tag="inv_bc")
            bc_ap = bass.AP(
                tensor=invrow.tensor,
                offset=invrow.offset,
                ap=[[0, P], [1, c_pad]],
            )
            nc.gpsimd.dma_start(inv_bc[:], bc_ap)

            # ---- pass 2 (group-outer): DMA w^T group, scale by inv-norm,
            # matmul all batch tiles, exp-accumulate ----
            sums = pb.tile([P, BO, NG], F32, tag="sums")
            grp_w = [sum(n_tiles[ct] for ct in grp) for grp in groups]
            grp_off = [0]
            for gw in grp_w:
                grp_off.append(grp_off[-1] + gw)
            for gi, grp in enumerate(groups):
                gw = grp_w[gi]
                c0 = grp_off[gi]
                Wg = pw.tile([P, KO, 2048], dt_mm, tag="Wg")
                nc.sync.dma_start(
                    Wg[:, :, :gw],
                    wt_d.ap()[:, KO * c0 : KO * (c0 + gw)].rearrange(
                        "p (k n) -> p k n", k=KO
                    ),
                )
                inv_b = inv_bc[:, None, c0 : c0 + gw].to_broadcast((P, KO, gw))
                nc.vector.tensor_tensor(
                    Wg[:, :, :gw], Wg[:, :, :gw], inv_b, ALU.mult
                )
                for bo in range(BO):
                    bs = slice(bo * P, (bo + 1) * P)
                    psm = pps.tile([P, 2048], F32, tag="ps")
                    if cfg["fp8_pairs"]:
                        for kp in range(KO // 2):
                            ks = slice(2 * kp, 2 * kp + 2)
                            for o in range(0, gw, 512):
                                nw = min(512, gw - o)
                                nc.tensor.matmul(
                                    psm[:, o : o + nw],
                                    et_sb[:, ks, bs],
                                    Wg[:, ks, o : o + nw],
                                    start=(kp == 0),
                                    stop=(kp == KO // 2 - 1),
                                    perf_mode=mybir.MatmulPerfMode.DoubleRow,
                                )
                    else:
                        for ko in range(KO):
                            for o in range(0, gw, 512):
                                nw = min(512, gw - o)
                                nc.tensor.matmul(
                                    psm[:, o : o + nw],
                                    et_sb[:, ko, bs],
                                    Wg[:, ko, o : o + nw],
                                    start=(ko == 0),
                                    stop=(ko == KO - 1),
                                )
                    scr = pscr.tile([P, 2048], BF16, tag="escr")
                    nc.scalar.activation(
                        scr[:, :gw],
                        psm[:, :gw],
                        AF.Exp,
                        scale=scale_act[:, bo : bo + 1],
                        accum_out=sums[:, bo, gi : gi + 1],
                    )

            # ---- combine: AllReduce of row-wise partial sums ----
            S_loc = ps.tile([P, BO], F32, tag="S_loc")
            nc.vector.reduce_sum(S_loc[:], sums[:], axis=AX.X)
            cc_in = pd.tile([P, BO], F32, tag="cc_in")
            cc_out = pd.tile([P, BO], F32, tag="cc_out")
            nc.gpsimd.dma_start(cc_in[:], S_loc[:])
            nc.gpsimd.collective_compute(
                "AllReduce",
                ALU.add,
                replica_groups=[list(range(n_cores))],
                ins=[cc_in.opt()],
                outs=[cc_out.opt()],
            )
            S_glob = ps.tile([P, BO], F32, tag="S_glob")
            nc.gpsimd.dma_start(S_glob[:], cc_out[:])

            # ---- finalize: S' = S - exp(l_t) + exp(l_m); loss = mean(ln S' - l_m)
            e_lt = ps.tile([P, BO], F32, tag="e_lt")
            e_lm = ps.tile([P, BO], F32, tag="e_lm")
            nc.scalar.activation(e_lt[:], l_t[:], AF.Exp)
            nc.scalar.activation(e_lm[:], l_m[:], AF.Exp)
            S2 = ps.tile([P, BO], F32, tag="S2")
            nc.vector.tensor_sub(S2[:], S_glob[:], e_lt[:])
            nc.vector.tensor_add(S2[:], S2[:], e_lm[:])
            lse = ps.tile([P, BO], F32, tag="lse")
            nc.scalar.activation(lse[:], S2[:], AF.Ln)
            per_b = ps.tile([P, BO], F32, tag="per_b")
            nc.vector.tensor_sub(per_b[:], lse[:], l_m[:])
            row = ps.tile([P, 1], F32, tag="row")
            nc.vector.reduce_sum(row[:], per_b[:], axis=AX.X)
            psf = pps.tile([1, 1], F32, tag="ps")
            nc.tensor.matmul(psf[:], ones_f[:], row[:], start=True, stop=True)
            loss_sb = ps.tile([1, 1], F32, tag="loss_sb")
            nc.scalar.mul(loss_sb[:], psf[:], 1.0 / b)
            nc.sync.dma_start(out_d.ap()[:], loss_sb[:])

    nc.compile()
    return nc


def prep_inputs(cfg, embeddings, weight, labels):
    """Shard + lay out the full inputs into per-core in_maps."""
    n_cores = cfg["n_cores"]
    b, d = cfg["b"], cfg["d"]
    c_local, c_pad = cfg["c_local"], cfg["c_pad"]
    np_mm = cfg["np_mm"]
    KO = d // 128
    BO = b // 128
    JP = c_pad // 128
    P = 128

    e = np.asarray(embeddings, np.float32)
    w = np.asarray(weight, np.float32)
    lab = np.asarray(labels).astype(np.int64)

    # replicated tensors
    et = (e.T * cfg["s_e"]).astype(np_mm)  # [d, b]
    et_host = np.ascontiguousarray(
        et.reshape(KO, P, b).transpose(1, 0, 2).reshape(P, KO * b)
    )
    e32_host = np.ascontiguousarray(
        e.reshape(BO, P, d).transpose(1, 0, 2).reshape(P, BO * d)
    )
    wl = w[lab]  # [b, d]
    wl32_host = np.ascontiguousarray(
        wl.reshape(BO, P, d).transpose(1, 0, 2).reshape(P, BO * d)
    )

    in_maps = []
    for i in range(n_cores):
        ws = w[i * c_local : (i + 1) * c_local]
        if c_pad > c_local:
            pad = np.zeros((c_pad - c_local, d), np.float32)
            pad[:, 0] = cfg["dummy_mag"] / cfg["s_w"]
            ws = np.concatenate([ws, pad], axis=0)
        ws_scaled = (ws * cfg["s_w"]).astype(np_mm)  # [c_pad, d]
        wt = ws_scaled.T  # [d, c_pad]
        wt4 = np.ascontiguousarray(wt).reshape(KO, P, c_pad)  # [ko, p, c]
        blocks = []
        c0 = 0
        for grp in cfg["groups"]:
            gw = sum(cfg["n_tiles"][ct] for ct in grp)
            blk = wt4[:, :, c0 : c0 + gw]  # [KO, P, gw]
            blocks.append(blk.transpose(1, 0, 2).reshape(P, KO * gw))
            c0 += gw
        wt_host = np.ascontiguousarray(np.concatenate(blocks, axis=1))
        # row-major copy for norms: partition p holds classes [p*JP,(p+1)*JP)
        wr_host = np.ascontiguousarray(ws_scaled.reshape(P, JP * d))
        in_maps.append(
            {
                "wt": wt_host,
                "wr": wr_host,
                "et": et_host,
                "e32": e32_host,
                "wl32": wl32_host,
            }
        )
    return in_maps


_CACHED = {}


def _get_nc(cfg_key, cfg):
    if cfg_key not in _CACHED:
        _CACHED[cfg_key] = build_nc(cfg)
    return _CACHED[cfg_key]


def run(inputs, mm_dtype="fp8", trace=False, **kw):
    from concourse.bass_utils import run_bass_kernel_spmd

    cfg = make_cfg(mm_dtype=mm_dtype)
    nc = _get_nc((mm_dtype,), cfg)
    in_maps = prep_inputs(
        cfg, inputs["embeddings"], inputs["weight"], inputs["labels"]
    )
    res = run_bass_kernel_spmd(
        nc, in_maps, core_ids=list(range(cfg["n_cores"])), trace=trace, **kw
    )
    loss = np.float32(res.results[0]["out"].reshape(-1)[0])
    return loss, res


def kernel(**inputs):
    loss, _ = run(inputs, trace=False)
    return np.asarray(loss, dtype=np.float32).reshape(())


# revision 15
# speedup vs baseline: 1.4335x; 1.0995x over previous
"""ArcFace loss on 8 TRN2 NeuronCores (Bass/Tile).

Strategy (model-parallel classification head):
  - Classes sharded across 8 cores (12500/core, padded to 12544).
  - Each core: cosine slice = e_hat @ w_hat_local^T on the TensorEngine
    (fp8 DoubleRow by default), with l2-normalization done on-device:
    per-class norms come from a one-pass DVE/ACT square-accumulate over a
    row-major copy of the shard, the inverse norms are folded into the
    weight tiles, and 1/||e|| is folded into the exp scale.
  - Row-wise sum of exp(SCALE * cosine): ACT exp with accum_out over
    2048-column PSUM groups (no max-stabilization needed: |logits| <= 64
    so exp fits comfortably in fp32).
  - AllReduce(add) of the per-row partial sums (4KB) across the 8 cores.
  - Target-class terms use host-gathered rows w[labels] (replicated,
    fp32) and cos(acos(x)+m) = x*cos(m) - sin(m)*sqrt(1-x^2); every core
    redundantly computes the final scalar.

kernel(**inputs) takes the FULL inputs and returns the full (scalar) output.
"""

import math

import numpy as np
import ml_dtypes

import concourse.bass as bass
import concourse.mybir as mybir
import concourse.tile as tile
from concourse import bacc

AF = mybir.ActivationFunctionType
ALU = mybir.AluOpType
AX = mybir.AxisListType
F32 = mybir.dt.float32
BF16 = mybir.dt.bfloat16

MARGIN = 0.5
SCALE = 64.0
EPS = 1e-7


def make_cfg(
    n_cores=8,
    b=1024,
    d=512,
    c_total=100000,
    mm_dtype="fp8",
):
    c_local = c_total // n_cores
    c_pad = ((c_local + 127) // 128) * 128
    n_tiles = []
    rem = c_pad
    while rem > 0:
        t = min(512, rem)
        n_tiles.append(t)
        rem -= t
    # groups of up to 4 column tiles -> one 2048-wide exp per group
    groups = []
    i = 0
    while i < len(n_tiles):
        g = list(range(i, min(i + 4, len(n_tiles))))
        if sum(n_tiles[j] for j in g) > 2048:
            g = g[:-1]
        groups.append(g)
        i = g[-1] + 1
    if mm_dtype == "bf16":
        dt_mm, np_mm = BF16, ml_dtypes.bfloat16
        s_w, s_e, g = 1.0, 1.0, 1.0
        fp8_pairs = False
    elif mm_dtype == "fp8":
        dt_mm, np_mm = mybir.dt.float8e4, ml_dtypes.float8_e4m3
        s_w, s_e, g = 128.0, 1.0, 16.0
        fp8_pairs = True
    else:
        raise ValueError(mm_dtype)
    return dict(
        n_cores=n_cores,
        b=b,
        d=d,
        c_total=c_total,
        c_local=c_local,
        c_pad=c_pad,
        n_tiles=n_tiles,
        groups=groups,
        dt_mm=dt_mm,
        np_mm=np_mm,
        s_w=s_w,
        s_e=s_e,
        g=g,
        fp8_pairs=fp8_pairs,
        dummy_mag=100.0,
    )


def build_nc(cfg):
    n_cores = cfg["n_cores"]
    b, d = cfg["b"], cfg["d"]
    c_pad = cfg["c_pad"]
    n_tiles = cfg["n_tiles"]
    groups = cfg["groups"]
    dt_mm = cfg["dt_mm"]
    g = cfg["g"]
    NT = len(n_tiles)
    NG = len(groups)
    KO = d // 128
    BO = b // 128
    JP = c_pad // 128
    P = 128

    nc = bacc.Bacc(
        "TRN2",
        target_bir_lowering=False,
        debug=False,
        enable_asserts=True,
        num_devices=n_cores,
    )

    wt_d = nc.dram_tensor("wt", [P, KO * c_pad], dt_mm, kind="ExternalInput")
    wr_d = nc.dram_tensor("wr", [P, JP * d], dt_mm, kind="ExternalInput")
    et_d = nc.dram_tensor("et", [P, KO * b], dt_mm, kind="ExternalInput")
    e32_d = nc.dram_tensor("e32", [P, BO * d], F32, kind="ExternalInput")
    wl32_d = nc.dram_tensor("wl32", [P, BO * d], F32, kind="ExternalInput")
    out_d = nc.dram_tensor("out", [1, 1], F32, kind="ExternalOutput")

    cos_m = math.cos(MARGIN)
    sin_m = math.sin(MARGIN)

    with tile.TileContext(nc) as tc:
        with (
            tc.tile_pool(name="const", bufs=1) as pc,
            tc.tile_pool(name="big", bufs=1) as pb,
            tc.tile_pool(name="wpool", bufs=NG) as pw,
            tc.tile_pool(name="wrpool", bufs=6) as pwr,
            tc.tile_pool(name="scr", bufs=3) as pscr,
            tc.tile_pool(name="nscr", bufs=4) as pnscr,
            tc.tile_pool(name="small", bufs=1) as ps,
            tc.tile_pool(name="ttrs", bufs=2) as pttr,
            tc.tile_pool(name="ps_all", bufs=2, space="PSUM") as pps,
            tc.tile_pool(name="dram", bufs=1, space="DRAM") as pd,
        ):
            # ---- constants ----
            ones_mm = pc.tile([P, P], BF16, tag="ones_mm")
            nc.vector.memset(ones_mm[:], 1.0)
            ones_f = pc.tile([P, 1], F32, tag="ones_f")
            nc.vector.memset(ones_f[:], 1.0)

            # ---- load replicated inputs ----
            et_sb = pb.tile([P, KO, b], dt_mm, tag="et")
            nc.sync.dma_start(
                et_sb[:], et_d.ap().rearrange("p (k b) -> p k b", k=KO)
            )
            # ---- norms of quantized embeddings (for the matmul path) ----
            sq_e = pb.tile([P, KO, b], BF16, tag="sq_e")
            nc.vector.tensor_tensor(sq_e[:], et_sb[:], et_sb[:], ALU.mult)
            ps_e = pps.tile([P, b], F32, tag="ps")
            for h0 in range(0, b, 512):
                hs = slice(h0, min(h0 + 512, b))
                for ko in range(KO):
                    nc.tensor.matmul(
                        ps_e[:, hs],
                        ones_mm[:],
                        sq_e[:, ko, hs],
                        start=(ko == 0),
                        stop=(ko == KO - 1),
                    )
            rne_b = ps.tile([P, b], F32, tag="rne_b")
            nc.vector.reciprocal(rne_b[:], ps_e[:])
            # reshape row 0 (replicated) [1, b] -> [P, BO] with b = bo*128 + p
            # via a DRAM bounce (SBUF partition dim can't be synthesized)
            scale_pre = ps.tile([P, BO], F32, tag="scale_pre")
            nrow = pd.tile([1, b], F32, tag="nrow")
            nc.sync.dma_start(nrow[:], rne_b[0:1, :])
            nc.sync.dma_start(
                scale_pre[:],
                nrow[0:1, :].rearrange("x (o p) -> p (x o)", p=P),
            )
            scale_act = ps.tile([P, BO], F32, tag="scale_act")
            # scale_act = (SCALE/g) / ||e_hat||  = sqrt((SCALE/g)^2 * (1/n2))
            nc.scalar.activation(
                scale_act[:], scale_pre[:], AF.Sqrt, scale=(SCALE / g) ** 2
            )

            # ---- pass 1: per-class norms via square-accumulate over the
            # row-major shard copy (compact [P, JP] layout, c = p*JP + j) ----
            nsq = ps.tile([P, JP], F32, tag="nsq")
            j = 0
            wg = 0
            while j < JP:
                gn = min(4, JP - j)
                wr_t = pwr.tile([P, 4, d], dt_mm, tag="wr")
                nc.sync.dma_start(
                    wr_t[:, :gn, :],
                    wr_d.ap()[:, j * d : (j + gn) * d].rearrange(
                        "p (j dd) -> p j dd", j=gn
                    ),
                )
                for jj in range(gn):
                    if (j + jj) % 2 == 0:
                        nscr = pnscr.tile([P, d], BF16, tag="nscr")
                        nc.scalar.activation(
                            nscr[:],
                            wr_t[:, jj, :],
                            AF.Square,
                            accum_out=nsq[:, j + jj : j + jj + 1],
                        )
                    else:
                        nscr = pnscr.tile([P, d], BF16, tag="nscr")
                        nc.vector.scalar_tensor_tensor(
                            out=nscr[:],
                            in0=wr_t[:, jj, :],
                            scalar=1.0,
                            in1=wr_t[:, jj, :],
                            op0=ALU.mult,
                            op1=ALU.mult,
                            accum_out=nsq[:, j + jj : j + jj + 1],
                        )
                j += gn
                wg += 1

            e32_sb = pb.tile([P, BO, d], F32, tag="e32")
            nc.sync.dma_start(
                e32_sb[:], e32_d.ap().rearrange("p (o d) -> p o d", o=BO)
            )
            wl32_sb = pb.tile([P, BO, d], F32, tag="wl32")
            nc.sync.dma_start(
                wl32_sb[:], wl32_d.ap().rearrange("p (o d) -> p o d", o=BO)
            )

            # ---- target path (fp32, reference-accurate) ----
            dot = ps.tile([P, BO], F32, tag="dot")
            ne2 = ps.tile([P, BO], F32, tag="ne2")
            nw2 = ps.tile([P, BO], F32, tag="nw2")
            for bo in range(BO):
                for dst, a, bb in (
                    (dot, e32_sb, wl32_sb),
                    (ne2, e32_sb, e32_sb),
                    (nw2, wl32_sb, wl32_sb),
                ):
                    scr = pttr.tile([P, d], F32, tag="ttr")
                    nc.vector.scalar_tensor_tensor(
                        out=scr[:],
                        in0=a[:, bo, :],
                        scalar=1.0,
                        in1=bb[:, bo, :],
                        op0=ALU.mult,
                        op1=ALU.mult,
                        accum_out=dst[:, bo : bo + 1],
                    )
            ne2r = ps.tile([P, BO], F32, tag="ne2r")
            nw2r = ps.tile([P, BO], F32, tag="nw2r")
            nc.vector.reciprocal(ne2r[:], ne2[:])
            nc.vector.reciprocal(nw2r[:], nw2[:])
            rne = ps.tile([P, BO], F32, tag="rne")
            rnw = ps.tile([P, BO], F32, tag="rnw")
            nc.scalar.activation(rne[:], ne2r[:], AF.Sqrt)
            nc.scalar.activation(rnw[:], nw2r[:], AF.Sqrt)
            cos_t = ps.tile([P, BO], F32, tag="cos_t")
            nc.vector.tensor_mul(cos_t[:], dot[:], rne[:])
            nc.vector.tensor_mul(cos_t[:], cos_t[:], rnw[:])
            cos_c = ps.tile([P, BO], F32, tag="cos_c")
            nc.vector.tensor_scalar(
                cos_c[:], cos_t[:], 1.0 - EPS, -1.0 + EPS, ALU.min, ALU.max
            )
            cs2 = ps.tile([P, BO], F32, tag="cs2")
            nc.vector.tensor_mul(cs2[:], cos_c[:], cos_c[:])
            sin_t = ps.tile([P, BO], F32, tag="sin_t")
            nc.scalar.activation(sin_t[:], cs2[:], AF.Sqrt, bias=1.0, scale=-1.0)
            tm1 = ps.tile([P, BO], F32, tag="tm1")
            tm2 = ps.tile([P, BO], F32, tag="tm2")
            nc.vector.tensor_scalar_mul(tm1[:], cos_c[:], cos_m)
            nc.vector.tensor_scalar_mul(tm2[:], sin_t[:], sin_m)
            tmod = ps.tile([P, BO], F32, tag="tmod")
            nc.vector.tensor_sub(tmod[:], tm1[:], tm2[:])
            l_m = ps.tile([P, BO], F32, tag="l_m")
            nc.vector.tensor_scalar_mul(l_m[:], tmod[:], SCALE)
            l_t = ps.tile([P, BO], F32, tag="l_t")
            nc.vector.tensor_scalar_mul(l_t[:], cos_t[:], SCALE)

            # inv_cc = g / ||W_c|| = sqrt(g^2 / n2), compact bf16
            nsqr = ps.tile([P, JP], F32, tag="nsqr")
            nc.vector.reciprocal(nsqr[:], nsq[:])
            inv_cc = ps.tile([P, JP], BF16, tag="inv_cc")
            nc.scalar.activation(inv_cc[:], nsqr[:], AF.Sqrt, scale=g * g)

            # compact [P, JP] -> DRAM row in class order (contiguous per
            # partition since c = p*JP + j) -> partition-broadcast back
            invrow = pd.tile([1, c_pad], BF16, tag="invrow")
            nc.sync.dma_start(
                invrow[0:1, :].rearrange("x (p j) -> p (x j)", p=P),
                inv_cc[:],
            )
            inv_bc = pb.tile([P, c_pad], BF16, tag="inv_bc")
            bc_ap = bass.AP(
                tensor=invrow.tensor,
                offset=invrow.offset,
                ap=[[0, P], [1, c_pad]],
            )
            nc.gpsimd.dma_start(inv_bc[:], bc_ap)

            # ---- pass 2 (group-outer): DMA w^T group, scale by inv-norm,
            # matmul all batch tiles, exp-accumulate ----
            sums = pb.tile([P, BO, NG], F32, tag="sums")
            grp_w = [sum(n_tiles[ct] for ct in grp) for grp in groups]
            grp_off = [0]
            for gw in grp_w:
                grp_off.append(grp_off[-1] + gw)
            w_tiles = {}

            def mains(bo_range):
                for gi, grp in enumerate(groups):
                    gw = grp_w[gi]
                    c0 = grp_off[gi]
                    if gi in w_tiles:
                        Wg = w_tiles[gi]
                    else:
                        Wg = pw.tile([P, KO, 2048], dt_mm, tag="Wg")
                        w_tiles[gi] = Wg
                        nc.sync.dma_start(
                            Wg[:, :, :gw],
                            wt_d.ap()[:, KO * c0 : KO * (c0 + gw)].rearrange(
                                "p (k n) -> p k n", k=KO
                            ),
                        )
                        inv_b = inv_bc[:, None, c0 : c0 + gw].to_broadcast(
                            (P, KO, gw)
                        )
                        nc.vector.tensor_tensor(
                            Wg[:, :, :gw], Wg[:, :, :gw], inv_b, ALU.mult
                        )
                    for bo in bo_range:
                        bs = slice(bo * P, (bo + 1) * P)
                        psm = pps.tile([P, 2048], F32, tag="ps")
                        if cfg["fp8_pairs"]:
                            for kp in range(KO // 2):
                                ks = slice(2 * kp, 2 * kp + 2)
                                for o in range(0, gw, 512):
                                    nw = min(512, gw - o)
                                    nc.tensor.matmul(
                                        psm[:, o : o + nw],
                                        et_sb[:, ks, bs],
                                        Wg[:, ks, o : o + nw],
                                        start=(kp == 0),
                                        stop=(kp == KO // 2 - 1),
                                        perf_mode=mybir.MatmulPerfMode.DoubleRow,
                                    )
                        else:
                            for ko in range(KO):
                                for o in range(0, gw, 512):
                                    nw = min(512, gw - o)
                                    nc.tensor.matmul(
                                        psm[:, o : o + nw],
                                        et_sb[:, ko, bs],
                                        Wg[:, ko, o : o + nw],
                                        start=(ko == 0),
                                        stop=(ko == KO - 1),
                                    )
                        scr = pscr.tile([P, 2048], BF16, tag="escr")
                        nc.scalar.activation(
                            scr[:, :gw],
                            psm[:, :gw],
                            AF.Exp,
                            scale=scale_act[:, bo : bo + 1],
                            accum_out=sums[:, bo, gi : gi + 1],
                        )

            # phase A (first half of the batch), then its AllReduce is
            # issued and overlaps phase B's compute
            half = BO // 2
            S_loc = ps.tile([P, BO], F32, tag="S_loc")
            S_glob = ps.tile([P, BO], F32, tag="S_glob")
            cc_in_a = pd.tile([P, half], F32, tag="cc_in_a")
            cc_out_a = pd.tile([P, half], F32, tag="cc_out_a")
            cc_in_b = pd.tile([P, BO - half], F32, tag="cc_in_b")
            cc_out_b = pd.tile([P, BO - half], F32, tag="cc_out_b")

            mains(range(half))
            nc.vector.reduce_sum(
                S_loc[:, 0:half], sums[:, 0:half, :], axis=AX.X
            )
            nc.gpsimd.dma_start(cc_in_a[:], S_loc[:, 0:half])
            nc.gpsimd.collective_compute(
                "AllReduce",
                ALU.add,
                replica_groups=[list(range(n_cores))],
                ins=[cc_in_a.opt()],
                outs=[cc_out_a.opt()],
            )
            nc.gpsimd.dma_start(S_glob[:, 0:half], cc_out_a[:])

            mains(range(half, BO))
            nc.vector.reduce_sum(
                S_loc[:, half:BO], sums[:, half:BO, :], axis=AX.X
            )
            nc.gpsimd.dma_start(cc_in_b[:], S_loc[:, half:BO])
            nc.gpsimd.collective_compute(
                "AllReduce",
                ALU.add,
                replica_groups=[list(range(n_cores))],
                ins=[cc_in_b.opt()],
                outs=[cc_out_b.opt()],
            )
            nc.gpsimd.dma_start(S_glob[:, half:BO], cc_out_b[:])

            # ---- finalize: S' = S - exp(l_t) + exp(l_m); loss = mean(ln S' - l_m)
            e_lt = ps.tile([P, BO], F32, tag="e_lt")
            e_lm = ps.tile([P, BO], F32, tag="e_lm")
            nc.scalar.activation(e_lt[:], l_t[:], AF.Exp)
            nc.scalar.activation(e_lm[:], l_m[:], AF.Exp)
            S2 = ps.tile([P, BO], F32, tag="S2")
            nc.vector.tensor_sub(S2[:], S_glob[:], e_lt[:])
            nc.vector.tensor_add(S2[:], S2[:], e_lm[:])
            lse = ps.tile([P, BO], F32, tag="lse")
            nc.scalar.activation(lse[:], S2[:], AF.Ln)
            per_b = ps.tile([P, BO], F32, tag="per_b")
            nc.vector.tensor_sub(per_b[:], lse[:], l_m[:])
            row = ps.tile([P, 1], F32, tag="row")
            nc.vector.reduce_sum(row[:], per_b[:], axis=AX.X)
            psf = pps.tile([1, 1], F32, tag="ps")
            nc.tensor.matmul(psf[:], ones_f[:], row[:], start=True, stop=True)
            loss_sb = ps.tile([1, 1], F32, tag="loss_sb")
            nc.scalar.mul(loss_sb[:], psf[:], 1.0 / b)
            nc.sync.dma_start(out_d.ap()[:], loss_sb[:])

    nc.compile()
    return nc


def prep_inputs(cfg, embeddings, weight, labels):
    """Shard + lay out the full inputs into per-core in_maps."""
    n_cores = cfg["n_cores"]
    b, d = cfg["b"], cfg["d"]
    c_local, c_pad = cfg["c_local"], cfg["c_pad"]
    np_mm = cfg["np_mm"]
    KO = d // 128
    BO = b // 128
    JP = c_pad // 128
    P = 128

    e = np.asarray(embeddings, np.float32)
    w = np.asarray(weight, np.float32)
    lab = np.asarray(labels).astype(np.int64)

    # replicated tensors
    et = (e.T * cfg["s_e"]).astype(np_mm)  # [d, b]
    et_host = np.ascontiguousarray(
        et.reshape(KO, P, b).transpose(1, 0, 2).reshape(P, KO * b)
    )
    e32_host = np.ascontiguousarray(
        e.reshape(BO, P, d).transpose(1, 0, 2).reshape(P, BO * d)
    )
    wl = w[lab]  # [b, d]
    wl32_host = np.ascontiguousarray(
        wl.reshape(BO, P, d).transpose(1, 0, 2).reshape(P, BO * d)
    )

    in_maps = []
    for i in range(n_cores):
        ws = w[i * c_local : (i + 1) * c_local]
        if c_pad > c_local:
            pad = np.zeros((c_pad - c_local, d), np.float32)
            pad[:, 0] = cfg["dummy_mag"] / cfg["s_w"]
            ws = np.concatenate([ws, pad], axis=0)
        ws_scaled = (ws * cfg["s_w"]).astype(np_mm)  # [c_pad, d]
        wt = ws_scaled.T  # [d, c_pad]
        wt4 = np.ascontiguousarray(wt).reshape(KO, P, c_pad)  # [ko, p, c]
        blocks = []
        c0 = 0
        for grp in cfg["groups"]:
            gw = sum(cfg["n_tiles"][ct] for ct in grp)
            blk = wt4[:, :, c0 : c0 + gw]  # [KO, P, gw]
            blocks.append(blk.transpose(1, 0, 2).reshape(P, KO * gw))
            c0 += gw
        wt_host = np.ascontiguousarray(np.concatenate(blocks, axis=1))
        # row-major copy for norms: partition p holds classes [p*JP,(p+1)*JP)
        wr_host = np.ascontiguousarray(ws_scaled.reshape(P, JP * d))
        in_maps.append(
            {
                "wt": wt_host,
                "wr": wr_host,
                "et": et_host,
                "e32": e32_host,
                "wl32": wl32_host,
            }
        )
    return in_maps


_CACHED = {}


def _get_nc(cfg_key, cfg):
    if cfg_key not in _CACHED:
        _CACHED[cfg_key] = build_nc(cfg)
    return _CACHED[cfg_key]


def run(inputs, mm_dtype="fp8", trace=False, **kw):
    from concourse.bass_utils import run_bass_kernel_spmd

    cfg = make_cfg(mm_dtype=mm_dtype)
    nc = _get_nc((mm_dtype,), cfg)
    in_maps = prep_inputs(
        cfg, inputs["embeddings"], inputs["weight"], inputs["labels"]
    )
    res = run_bass_kernel_spmd(
        nc, in_maps, core_ids=list(range(cfg["n_cores"])), trace=trace, **kw
    )
    loss = np.float32(res.results[0]["out"].reshape(-1)[0])
    return loss, res


def kernel(**inputs):
    loss, _ = run(inputs, trace=False)
    return np.asarray(loss, dtype=np.float32).reshape(())
